# revision 74
# baseline (speedup 1.0000x reference)
"""AxisAttention TRN2 Bass kernel.

Full-input contract: kernel(**inputs) takes the unsharded numpy inputs and
returns the full [4, 2048, 512] float32 output.

Sharding: data-parallel over (batch, query-half) -> 8 NeuronCores. Each core
computes attention for 1024 queries of one batch against that batch's full
2048 keys. Params are replicated. K/V projections are recomputed by the two
cores sharing a batch (cheaper than a cross-core exchange).

Key structure (n=1024 queries, m=2048 keys, d=a=c=512):
  Wvo     = Wv @ Wo folded on host: out = (P/rsum) @ (kv @ Wvo) + query,
            which removes the whole output-projection phase on the PE.
  qT[a,n] = sum_d WqS[d,a] * xqT[d,n]          (WqS = Wq*sqrt(512), fp16)
  kT[a,m] = sum_d Wk[d,a] * xkvT[d,m]
  v8      = (kv8 @ Wvo8) via fp8 DoubleRow matmuls (host packs kv^T and Wvo
            into the [K/256, 128, 2, .] d-pair layout), out fp8 e4m3 stored
            [128, 16, 512]: m-tile pairs = the DoubleRow block layout.
  S[n,m]  = sum_a qT[a,n] * kT[a,m]   (PSUM f32, 1024-col halves -- moving
            operand at its 1024-element max to amortize per-matmul overhead)
  P[n,m]  = exp(S - rowmax(S)) fp16; rowsum via ACT accum
  PTB     = fp16 DMA-xbar transpose of P per half; PT8 = fp8 cast of PTB
            split across DVE/ACT/GpSimd so it never gates the PE
  O[n,c]  = sum_m P[n,m] * v'[m,c] as 8 fp8 DoubleRow matmuls (K=256 each,
            ~2x PE throughput) per query tile, trailing the score pipeline
  out[n,:]= O * (1/rowsum)[n] + query32[n,:]   (+bvo broadcast if nonzero)

fp16 matmul operands for q/k/scores (the softmax is near-argmax at this
score scale, so q/k noise flips argmaxes and must stay small); fp8 e4m3 for
the value path and P*V contraction (the attention term is ~20% of the result
norm, so ~4% fp8 noise lands near 9e-3 relative error, under the 2e-2 gate;
Wvo is pre-scaled by 32 so its ~0.009-std entries stay out of e4m3's
subnormal range). PV and the store pipeline interleave with the score tiles,
so there is no output tail. Input loads alternate between the two HWDGE
queues. Measured: ~113 us HW exec (traced), from a 152.6 us traced baseline.
"""

import numpy as np
import ml_dtypes

import concourse.bass as bass
import concourse.mybir as mybir
import concourse.tile as tile
from concourse import bacc
from concourse.bass_utils import run_bass_kernel_spmd

F8 = mybir.dt.float8e4
F16 = mybir.dt.float16
F32 = mybir.dt.float32
AX = mybir.AxisListType
ALU = mybir.AluOpType
ACTF = mybir.ActivationFunctionType
DR = mybir.MatmulPerfMode.DoubleRow
NP_F8 = ml_dtypes.float8_e4m3

B, N, D = 4, 2048, 512
N_CORES = 8
NQ = N // 2          # 1024 queries per core
M = N                # 2048 keys per core
P = 128              # partitions
SCALE = float(np.sqrt(float(D)))

ND = D // P          # 4 contraction chunks of 128
NDP = D // 256       # 2 d-pair chunks for fp8 DoubleRow
NNT = NQ // P        # 8 query tiles of 128
NMT = M // P         # 16 key tiles of 128
NMH = M // 1024      # 2 key halves of 1024
NB = M // 256        # 8 DoubleRow blocks (K=256 each)

PV_LAG = 3           # PV trails the score pipeline by this many tiles


def _sl(i, w=P):
    return slice(i * w, (i + 1) * w)


def _build(with_bqk: bool, with_bvo: bool):
    nc = bacc.Bacc("TRN2", target_bir_lowering=False, debug=False,
                   num_devices=N_CORES)

    xqT16 = nc.dram_tensor("xqT16", [D, NQ], F16, kind="ExternalInput").ap()
    xkvT16 = nc.dram_tensor("xkvT16", [D, M], F16, kind="ExternalInput").ap()
    wvo8d = nc.dram_tensor("wvo8", [2 * P, 2, D], F8, kind="ExternalInput").ap()
    xq32 = nc.dram_tensor("xq32", [NQ, D], F32, kind="ExternalInput").ap()
    wq = nc.dram_tensor("wq16", [D, D], F16, kind="ExternalInput").ap()
    wk = nc.dram_tensor("wk16", [D, D], F16, kind="ExternalInput").ap()
    if with_bqk:
        bq = nc.dram_tensor("bq", [D], F32, kind="ExternalInput").ap()
        bk = nc.dram_tensor("bk", [D], F32, kind="ExternalInput").ap()
    if with_bvo:
        bvo = nc.dram_tensor("bvo32", [1, D], F32, kind="ExternalInput").ap()
    out = nc.dram_tensor("out", [NQ, D], F16, kind="ExternalOutput").ap()

    # alternate input loads between the two HWDGE queues (SP + ACT)
    _ldq = [0]

    def load(dst, src):
        eng = nc.sync if _ldq[0] % 2 == 0 else nc.scalar
        _ldq[0] += 1
        eng.dma_start(out=dst, in_=src)

    with tile.TileContext(nc) as tc:
        with tc.tile_pool(name="pers", bufs=1) as pers:
            # ---- constant loads, in consumption order ----------------------
            # weight tiles hold all 4 d-chunks: one 3D-AP DMA each instead of
            # four small transfers (saves startup queue overhead)
            WQa = pers.tile([P, ND, D], F16, name="wqa", tag="wqa")
            WKa = pers.tile([P, ND, D], F16, name="wka", tag="wka")
            WQ = [WQa[:, d, :] for d in range(ND)]
            WK = [WKa[:, d, :] for d in range(ND)]
            XQT = [pers.tile([P, NQ], F16, name=f"xqt{d}", tag=f"xqt{d}") for d in range(ND)]
            XKVT = [pers.tile([P, M], F16, name=f"xkvt{d}", tag=f"xkvt{d}") for d in range(ND)]
            XKV8 = [pers.tile([P, 2, M], F8, name=f"xkv8_{p}", tag=f"xkv8_{p}") for p in range(NDP)]
            WVO8 = [pers.tile([P, 2, D], F8, name=f"wvo8_{p}", tag=f"wvo8_{p}") for p in range(NDP)]
            XQ32 = [pers.tile([P, D], F32, name=f"xq32_{t}", tag=f"xq32_{t}") for t in range(NNT)]
            # loads in consumption order: the k projection runs first, so
            # WK[d] + XKVT[d] pairs lead; q-path loads fill in behind while
            # the k/v projections compute
            # XKVT loads are d-major whole tiles to match the d-major matmul
            # consumption order (the first k-proj group runs d0 x c0..c3, so
            # it needs ALL of XKVT[0] first, not c0 of every d)
            # first-consumed pieces lead at fine granularity so the first
            # k-proj matmuls start after ~384KB, not ~1MB, of transfers
            wkr = wk.rearrange("(d p) c -> p d c", p=P)
            load(WKa[:, 0:1, :], wkr[:, 0:1, :])
            load(XKVT[0][:, _sl(0, 1024)], xkvT16[_sl(0), _sl(0, 1024)])
            load(WKa[:, 1:ND, :], wkr[:, 1:ND, :])
            load(XKVT[0][:, _sl(1, 1024)], xkvT16[_sl(0), _sl(1, 1024)])
            for d in range(1, ND):
                load(XKVT[d][:, _sl(0, 1024)], xkvT16[_sl(d), _sl(0, 1024)])
                load(XKVT[d][:, _sl(1, 1024)], xkvT16[_sl(d), _sl(1, 1024)])
            for p in range(NDP):
                load(WVO8[p][:], wvo8d[_sl(p), :, :])
            load(WQa[:], wq.rearrange("(d p) c -> p d c", p=P))
            for d in range(ND):
                load(XQT[d][:], xqT16[_sl(d), :])
            for t in range(NNT):
                load(XQ32[t][:], xq32[_sl(t), :])
            if with_bqk:
                BQ = [pers.tile([P, 1], F32, name=f"bq{i}", tag=f"bq{i}") for i in range(ND)]
                BK = [pers.tile([P, 1], F32, name=f"bk{i}", tag=f"bk{i}") for i in range(ND)]
                for i in range(ND):
                    nc.sync.dma_start(out=BQ[i][:],
                                      in_=bq[_sl(i)].rearrange("(a b) -> a b", b=1))
                    nc.sync.dma_start(out=BK[i][:],
                                      in_=bk[_sl(i)].rearrange("(a b) -> a b", b=1))
            if with_bvo:
                BVO = pers.tile([1, D], F32, name="bvo", tag="bvo")
                BVOB = pers.tile([P, D], F32, name="bvob", tag="bvob")
                nc.sync.dma_start(out=BVO[:], in_=bvo[:])
                nc.gpsimd.partition_broadcast(BVOB[:], BVO[:])

            # ---- projections ----------------------------------------------
            qT = [pers.tile([P, NQ], F16, name=f"qT{a}", tag=f"qT{a}") for a in range(ND)]
            kT = [pers.tile([P, M], F16, name=f"kT{a}", tag=f"kT{a}") for a in range(ND)]
            # v' = kv @ Wvo as fp8, [j, mt, c]: m = 128*mt + j (DoubleRow
            # pair = adjacent m-tiles, pair step 512 B)
            v8 = pers.tile([P, NMT, D], F8, name="v8", tag="v8")
            # fp16 transpose of P (m on partitions), then fp8 cast of it
            PTB = pers.tile([P, NMT, NQ], F16, name="PTB", tag="PTB")
            PT8 = pers.tile([P, NMT, NQ], F8, name="PT8", tag="PT8")
            recip = [pers.tile([P, 2], F32, name=f"recip{t}", tag=f"recip{t}") for t in range(NNT)]

            # d-pair fp8 copies of kv^T for the DoubleRow v-projection,
            # cast on-chip (cheaper than loading a second copy from DRAM)
            for p in range(NDP):
                nc.vector.tensor_copy(XKV8[p][:, 0, :], XKVT[2 * p][:])
                nc.scalar.copy(XKV8[p][:, 1, :], XKVT[2 * p + 1][:])

            with tc.tile_pool(name="pps", bufs=8, space="PSUM") as pps:
                for a in range(ND):
                    pss = [pps.tile([P, 512], F32, name="projps", tag="projps")
                           for _ in range(4)]
                    for d in range(ND):
                        for c in range(4):
                            nc.tensor.matmul(pss[c][:], WK[d][:, _sl(a)],
                                             XKVT[d][:, _sl(c, 512)],
                                             start=(d == 0), stop=(d == ND - 1))
                    for c in range(4):
                        if with_bqk:
                            nc.vector.tensor_scalar_add(
                                kT[a][:, _sl(c, 512)], pss[c][:], BK[a][:])
                        elif c % 2 == 0:
                            nc.vector.tensor_copy(kT[a][:, _sl(c, 512)],
                                                  pss[c][:])
                        else:
                            nc.scalar.copy(kT[a][:, _sl(c, 512)], pss[c][:])
                # v' projection: fp8 DoubleRow, out [m-tile 128, c 512].
                # Wvo is pre-scaled by 32 on the host so its entries sit in
                # e4m3's normal range (raw std ~0.009 is subnormal there);
                # the 1/32 rides the psum->fp8 copy for free.
                for mt in range(NMT):
                    ps = pps.tile([P, D], F32, name="projps", tag="projps")
                    for p in range(NDP):
                        nc.tensor.matmul(ps[:], XKV8[p][:, :, _sl(mt)],
                                         WVO8[p][:],
                                         start=(p == 0), stop=(p == NDP - 1),
                                         perf_mode=DR)
                    nc.scalar.activation(v8[:, mt, :], ps[:], ACTF.Copy,
                                         scale=1.0 / 32.0)
                for a in range(ND):
                    pss = [pps.tile([P, 512], F32, name="projps", tag="projps")
                           for _ in range(2)]
                    for d in range(ND):
                        for c in range(2):
                            nc.tensor.matmul(pss[c][:], WQ[d][:, _sl(a)],
                                             XQT[d][:, _sl(c, 512)],
                                             start=(d == 0), stop=(d == ND - 1))
                    for c in range(2):
                        if with_bqk:
                            nc.vector.tensor_scalar_add(
                                qT[a][:, _sl(c, 512)], pss[c][:], BQ[a][:])
                        else:
                            nc.vector.tensor_copy(qT[a][:, _sl(c, 512)],
                                                  pss[c][:])

            # ---- scores + softmax + PV + store, per query tile ------------
            with tc.tile_pool(name="spool", bufs=3, space="PSUM") as spool, \
                 tc.tile_pool(name="pvps", bufs=2, space="PSUM") as pvps, \
                 tc.tile_pool(name="ppool", bufs=3) as ppool, \
                 tc.tile_pool(name="stat", bufs=6) as stat, \
                 tc.tile_pool(name="fin", bufs=4) as fin:

                def score_tile(t):
                    # flash-style split: each 1024-col half is softmaxed
                    # against its own rowmax so the exp/transpose/cast chain
                    # for half A fires mid-tile; the halves are merged by
                    # per-row scalars in the final combine. Each half is one
                    # 2-bank PSUM tile: a single reduce, a single exp and a
                    # single accumulator read per half (the exp could never
                    # fire before the whole half's matmuls anyway).
                    nmh = stat.tile([P, 2], F32, name="nmh", tag="nmh")
                    rsh = stat.tile([P, 2], F32, name="rsh", tag="rsh")
                    pt = ppool.tile([P, M], F16, name="P", tag="P")
                    for h in range(2):
                        shp = spool.tile([P, 1024], F32, name="S", tag="S")
                        for c in range(2):
                            for a in range(ND):
                                nc.tensor.matmul(shp[:, _sl(c, 512)],
                                                 qT[a][:, _sl(t)],
                                                 kT[a][:, _sl(2 * h + c, 512)],
                                                 start=(a == 0),
                                                 stop=(a == ND - 1))
                        nc.vector.tensor_reduce(nmh[:, h:h + 1], shp[:],
                                                axis=AX.X, op=ALU.max,
                                                negate=True)
                        nc.scalar.activation(pt[:, _sl(h, 1024)], shp[:],
                                             ACTF.Exp, bias=nmh[:, h:h + 1],
                                             scale=1.0,
                                             accum_out=rsh[:, h:h + 1])
                        hmt = slice(8 * h, 8 * h + 8)
                        nc.sync.dma_start(out=PTB[:, hmt, _sl(t)],
                                          in_=pt[:, _sl(h, 1024)],
                                          transpose=True)
                    # merge scalars: recip[t][:,h] = exp(negmax - nmh)/rowsum;
                    # rowsum = sum_h rsh*exp(negmax - nmh). All [128,1] ops,
                    # issued ahead of the casts so the PV combine never waits.
                    negmax = stat.tile([P, 1], F32, name="negmax", tag="negmax")
                    nc.vector.tensor_reduce(negmax[:], nmh[:], axis=AX.X,
                                            op=ALU.min)
                    # eh = nmh - negmax >= 0;  sh = exp(-eh)
                    eh = stat.tile([P, 2], F32, name="eh", tag="eh")
                    nc.vector.tensor_scalar(eh[:], nmh[:], negmax[:], None,
                                            op0=ALU.subtract)
                    sh = stat.tile([P, 2], F32, name="sh", tag="sh")
                    nc.scalar.activation(sh[:], eh[:], ACTF.Exp, scale=-1.0)
                    wrs = stat.tile([P, 2], F32, name="wrs", tag="wrs")
                    nc.vector.tensor_tensor(wrs[:], rsh[:], sh[:], op=ALU.mult)
                    rowsum = stat.tile([P, 1], F32, name="rowsum", tag="rowsum")
                    nc.vector.tensor_reduce(rowsum[:], wrs[:], axis=AX.X,
                                            op=ALU.add)
                    rr = stat.tile([P, 1], F32, name="rr", tag="rr")
                    nc.vector.reciprocal(rr[:], rowsum[:])
                    nc.vector.tensor_scalar(recip[t][:], sh[:], rr[:],
                                            None, op0=ALU.mult)
                    # fp8 casts of the transposed P, split across engines
                    for h in range(2):
                        mt0 = 8 * h
                        nc.vector.tensor_copy(
                            PT8[:, mt0:mt0 + 3, _sl(t)],
                            PTB[:, mt0:mt0 + 3, _sl(t)])
                        nc.scalar.copy(
                            PT8[:, mt0 + 3:mt0 + 7, _sl(t)],
                            PTB[:, mt0 + 3:mt0 + 7, _sl(t)])
                        nc.gpsimd.tensor_copy(
                            PT8[:, mt0 + 7:mt0 + 8, _sl(t)],
                            PTB[:, mt0 + 7:mt0 + 8, _sl(t)])

                def pv_tile(t):
                    psh = []
                    for h in range(2):
                        ps = pvps.tile([P, D], F32, name="pv", tag="pv")
                        for b in range(4 * h, 4 * h + 4):
                            nc.tensor.matmul(ps[:],
                                             PT8[:, 2 * b:2 * b + 2, _sl(t)],
                                             v8[:, 2 * b:2 * b + 2, :],
                                             start=(b == 4 * h),
                                             stop=(b == 4 * h + 3),
                                             perf_mode=DR)
                        psh.append(ps)
                    osb = fin.tile([P, D], F32, name="osb", tag="osb")
                    nc.vector.scalar_tensor_tensor(
                        out=osb[:], in0=psh[0][:], scalar=recip[t][:, 0:1],
                        in1=XQ32[t][:], op0=ALU.mult, op1=ALU.add)
                    osb2 = fin.tile([P, D], F16, name="osb2", tag="osb2")
                    nc.vector.scalar_tensor_tensor(
                        out=osb2[:], in0=psh[1][:], scalar=recip[t][:, 1:2],
                        in1=osb[:], op0=ALU.mult, op1=ALU.add)
                    if with_bvo:
                        nc.vector.tensor_add(osb2[:], osb2[:], BVOB[:])
                    nc.sync.dma_start(out=out[_sl(t), :], in_=osb2[:])

                for step in range(NNT + PV_LAG):
                    if step < NNT:
                        score_tile(step)
                    if step >= PV_LAG:
                        pv_tile(step - PV_LAG)

    nc.compile()
    return nc


_BUILD_CACHE = {}


def _get_nc(with_bqk: bool, with_bvo: bool):
    key = (with_bqk, with_bvo)
    if key not in _BUILD_CACHE:
        _BUILD_CACHE[key] = _build(with_bqk, with_bvo)
    return _BUILD_CACHE[key]


def kernel(query, key_value, Wq, bq, Wk, bk, Wv, bv, Wo, bo, _timing=None):
    query = np.asarray(query, dtype=np.float32)
    key_value = np.asarray(key_value, dtype=np.float32)
    Wq = np.asarray(Wq, dtype=np.float32)
    Wk = np.asarray(Wk, dtype=np.float32)
    Wv = np.asarray(Wv, dtype=np.float32)
    Wo = np.asarray(Wo, dtype=np.float32)
    bq = np.asarray(bq, dtype=np.float32)
    bk = np.asarray(bk, dtype=np.float32)
    bv = np.asarray(bv, dtype=np.float32)
    bo = np.asarray(bo, dtype=np.float32)

    # fold the output projection into the value projection
    Wvo = (Wv @ Wo).astype(np.float32)
    bvo = (bv @ Wo + bo).astype(np.float32)

    with_bqk = bool(np.any(bq)) or bool(np.any(bk))
    with_bvo = bool(np.any(bvo))
    nc = _get_nc(with_bqk, with_bvo)

    wq16 = (Wq * SCALE).astype(np.float16)
    wk16 = Wk.astype(np.float16)
    bqs = (bq * SCALE).astype(np.float32)
    bk32 = bk.astype(np.float32)
    bvo32 = bvo.reshape(1, D)

    q16 = query.astype(np.float16)
    kv16 = key_value.astype(np.float16)

    # d-pair (DoubleRow block) layouts: [dp*128+j, i, .] = src[256dp+128i+j, .]
    # Wvo * 32 keeps the entries in e4m3's normal range; the kernel applies
    # the 1/32 in the psum->fp8 copy after the v' projection.
    wvo8 = np.ascontiguousarray(
        (Wvo * 32.0).reshape(2, 2, P, D).transpose(0, 2, 1, 3)
        .reshape(2 * P, 2, D)
    ).astype(NP_F8)

    in_maps = []
    for core in range(N_CORES):
        b, h = divmod(core, 2)
        sl = slice(h * NQ, (h + 1) * NQ)
        im = {
            "xqT16": np.ascontiguousarray(q16[b, sl].T),
            "xkvT16": np.ascontiguousarray(kv16[b].T),
            "wvo8": wvo8,
            "xq32": np.ascontiguousarray(query[b, sl]),
            "wq16": wq16, "wk16": wk16,
        }
        if with_bqk:
            im["bq"] = bqs
            im["bk"] = bk32
        if with_bvo:
            im["bvo32"] = bvo32
        in_maps.append(im)

    res = run_bass_kernel_spmd(nc, in_maps, list(range(N_CORES)),
                               **(_timing or {}))
    out = np.empty((B, N, D), dtype=np.float32)
    for core in range(N_CORES):
        b, h = divmod(core, 2)
        out[b, h * NQ:(h + 1) * NQ] = res.results[core]["out"].astype(np.float32)
    if _timing is not None:
        return out, res
    return out
